# revision 3
# baseline (speedup 1.0000x reference)
"""LDPC neural BP decoder on 8 Trainium2 cores (Bass/Tile) — v2.

Data-parallel: batch 1024 -> 8 cores x 128 rows (batch on SBUF partitions).

Structure per iteration (5 total), chunked by 1056 nodes (8 chunks):
  APGather call per chunk: all K=10 slices k-major (num_idxs=10560 >= 8448
    input floor -> full Pool efficiency ~1.39ns/elem)
  DVE: pair-product chain (f32 pair muls -> fp16 partials), final mul + clip f32
  ACT: ln1=Ln(1+p), ln2=Ln(1-p)
  PE:  fp32r identity matmuls accumulate illr + var + varp + ln1 - ln2 in PSUM
       (3 sub-psums of 352 cols per chunk)
  ACT: evacuate PSUM -> var chunk (Copy) and t chunk (Tanh scale=0.5)
  DMA: stream illr/varp chunks from HBM (SBUF pressure), varp ping-pong buffers
Last iteration: one extra illr matmul then Sigmoid(psum) -> y.
Specialized for w_ch == w_res == 1 (spec fill=ones); otherwise falls back
to a host (numpy) computation which is always correct.
"""

import sys

sys.path.insert(0, "/opt/trn_rl_repo")

import numpy as np

NUM_ITERATIONS = 5
N = 8448
K = 10
B = 1024
NCORES = 8
BCORE = B // NCORES  # 128 = SBUF partitions
NC = 1056  # nodes per chunk
NCHUNK = N // NC  # 8
GI = NC * K  # gather indices per chunk (10560)
PC = 352  # psum sub-chunk cols
NPC = NC // PC  # 3


def _host_reference(input_llr, w_ch, w_res, check_index_tensor):
    x = input_llr.astype(np.float32)
    wl = x * w_ch[None, :].astype(np.float32)
    var = wl.copy()
    prev = []
    for _ in range(NUM_ITERATIONS):
        prev.insert(0, var)
        prev = prev[:2]
        g = var[:, check_index_tensor]
        t = np.tanh(np.clip(0.5 * g, -9.9, 9.9))
        p = np.clip(np.prod(t, axis=-1), -0.999999, 0.999999)
        cm = 2.0 * np.arctanh(p)
        res = sum(w_res[l][None, :] * prev[l] for l in range(len(prev)))
        var = wl + cm + res
    return (1.0 / (1.0 + np.exp(-(var + x)))).astype(np.float32)


def _build_nc():
    import concourse.bacc as bacc
    import concourse.mybir as mybir
    from concourse.tile import TileContext

    AF = mybir.ActivationFunctionType
    f32 = mybir.dt.float32
    f32r = mybir.dt.float32r
    f16 = mybir.dt.float16
    i16 = mybir.dt.int16

    nc = bacc.Bacc("TRN2", target_bir_lowering=False, debug=False)
    x_d = nc.dram_tensor("x", [BCORE, N], f32, kind="ExternalInput")
    idx_d = nc.dram_tensor("idx", [128, (N * K) // 16], i16, kind="ExternalInput")
    id_d = nc.dram_tensor("ident", [128, 128], f32, kind="ExternalInput")
    nid_d = nc.dram_tensor("nident", [128, 128], f32, kind="ExternalInput")
    y_d = nc.dram_tensor("y", [BCORE, N], f32, kind="ExternalOutput")
    # varp ping-pong (holds var_{i-1} while var tile holds var_i)
    vp_a = nc.dram_tensor("vp_a", [BCORE, N], f32, kind="Internal")
    vp_b = nc.dram_tensor("vp_b", [BCORE, N], f32, kind="Internal")
    vps = [vp_a, vp_b]

    ICOLS = GI // 16  # idx cols per chunk (660)

    with TileContext(nc) as tc:
        with (
            tc.tile_pool(name="persist", bufs=1) as pp,
            tc.tile_pool(name="gath", bufs=2) as gp,
            tc.tile_pool(name="strm", bufs=2) as sp_,
            tc.tile_pool(name="work", bufs=1) as wp,
            tc.tile_pool(name="workr", bufs=2) as wpr,
            tc.tile_pool(name="psum", bufs=6, space="PSUM") as psp,
        ):
            var = pp.tile([128, N], f32r, tag="var")
            t = pp.tile([128, N], f32, tag="t")
            idx = pp.tile([128, (N * K) // 16], i16, tag="idx")
            ident = pp.tile([128, 128], f32r, tag="ident")
            nident = pp.tile([128, 128], f32r, tag="nident")

            nc.sync.dma_start(idx[:], idx_d[:])
            nc.sync.dma_start(ident[:], id_d[:].bitcast(f32r))
            nc.sync.dma_start(nident[:], nid_d[:].bitcast(f32r))

            # t = tanh(0.5 * illr); var tile is first written by iter-0 evac.
            # use a gather buf as scratch for the initial illr load
            g0 = gp.tile([128, GI], f32, tag="g")
            H2 = N // 2
            nc.sync.dma_start(g0[:, 0:H2], x_d[:, 0:H2])
            nc.scalar.activation(t[:, 0:H2], g0[:, 0:H2], AF.Tanh, scale=0.5)
            nc.sync.dma_start(g0[:, H2:N], x_d[:, H2:N])
            nc.scalar.activation(t[:, H2:N], g0[:, H2:N], AF.Tanh, scale=0.5)

            for it in range(NUM_ITERATIONS):
                last = it == NUM_ITERATIONS - 1
                for c in range(NCHUNK):
                    cs = slice(c * NC, (c + 1) * NC)
                    g = gp.tile([128, GI], f32, tag="g")
                    nc.gpsimd.ap_gather(
                        g[:],
                        t[:],
                        idx[:, c * ICOLS : (c + 1) * ICOLS],
                        channels=128,
                        num_elems=N,
                        d=1,
                        num_idxs=GI,
                    )
                    def gk(k):
                        return g[:, k * NC : (k + 1) * NC]

                    s0 = wp.tile([128, NC], f16, tag="s0")
                    s1 = wp.tile([128, NC], f16, tag="s1")
                    p32 = wp.tile([128, NC], f32, tag="p32")
                    ln1 = wpr.tile([128, NC], f32r, tag="ln1")
                    ln2 = wpr.tile([128, NC], f32r, tag="ln2")
                    # product chain: fp16 partials, f32 pair muls. The last
                    # chunk is processed in NPC sub-slices so the serial
                    # iteration tail after the final gather is ~1/3 as long.
                    subs = (
                        [slice(i * PC, (i + 1) * PC) for i in range(NPC)]
                        if c == NCHUNK - 1
                        else [slice(0, NC)]
                    )
                    for sb in subs:
                        nc.vector.tensor_mul(s0[sb_p := (slice(None), sb)][...] if False else s0[:, sb], gk(0)[:, sb], gk(1)[:, sb])
                        nc.vector.tensor_mul(s1[:, sb], gk(2)[:, sb], gk(3)[:, sb])
                        nc.vector.tensor_mul(s0[:, sb], s0[:, sb], s1[:, sb])
                        nc.vector.tensor_mul(s1[:, sb], gk(4)[:, sb], gk(5)[:, sb])
                        nc.vector.tensor_mul(s0[:, sb], s0[:, sb], s1[:, sb])
                        nc.vector.tensor_mul(s1[:, sb], gk(6)[:, sb], gk(7)[:, sb])
                        nc.vector.tensor_mul(s0[:, sb], s0[:, sb], s1[:, sb])
                        nc.vector.tensor_mul(s1[:, sb], gk(8)[:, sb], gk(9)[:, sb])
                        nc.vector.tensor_mul(p32[:, sb], s0[:, sb], s1[:, sb])
                        nc.vector.tensor_scalar(
                            p32[:, sb], p32[:, sb], 0.999999, -0.999999,
                            op0=mybir.AluOpType.min, op1=mybir.AluOpType.max,
                        )
                        nc.scalar.activation(ln1[:, sb], p32[:, sb], AF.Ln, bias=1.0, scale=1.0)
                        nc.scalar.activation(ln2[:, sb], p32[:, sb], AF.Ln, bias=1.0, scale=-1.0)
                    if c == NCHUNK - 1:
                        # all gathers + lns of this iteration are issued: the
                        # bulk of next iteration's t (chunks 0..6, already
                        # evacuated) can be activated now, overlapping the
                        # last chunk's sum pipeline. Deprioritized so the
                        # scheduler runs the last chunk's Ln pair first (one
                        # Tanh table load in the tail instead of several).
                        h = (NCHUNK - 1) * NC
                        with tc.high_priority():
                            if last:
                                nc.scalar.activation(
                                    t[:, 0:h], var[:, 0:h].bitcast(f32),
                                    AF.Sigmoid,
                                )
                            else:
                                nc.scalar.activation(
                                    t[:, 0:h], var[:, 0:h].bitcast(f32),
                                    AF.Tanh, scale=0.5,
                                )

                    illr_c = sp_.tile([128, NC], f32r, tag="illr_c")
                    nc.sync.dma_start(illr_c[:], x_d[:, cs].bitcast(f32r))
                    if it >= 1:
                        varp_c = sp_.tile([128, NC], f32r, tag="varp_c")
                        src = x_d if it == 1 else vps[it % 2]
                        nc.sync.dma_start(varp_c[:], src[:, cs].bitcast(f32r))
                    if 1 <= it < NUM_ITERATIONS - 1:
                        # stash var_i chunk for iter i+1's varp (reads var_c
                        # before evac overwrites it below); iter 1 reads x_d
                        # directly so iter 0 has nothing to stash
                        nc.sync.dma_start(vps[(it + 1) % 2][:, cs].bitcast(f32r), var[:, cs])

                    for s in range(NPC):
                        ps_ = slice(s * PC, (s + 1) * PC)
                        vs = slice(c * NC + s * PC, c * NC + (s + 1) * PC)
                        ps = psp.tile([128, PC], f32, tag="ps")
                        # accumulation term list: (lhsT, rhs)
                        terms = [
                            (ident, ln1[:, ps_]),
                            (nident, ln2[:, ps_]),
                            (ident, illr_c[:, ps_]),
                        ]
                        if it == 0:
                            terms.append((ident, illr_c[:, ps_]))  # var_0 = illr
                        else:
                            terms.append((ident, var[:, vs]))
                            terms.append((ident, varp_c[:, ps_]))
                        if last:
                            # var_5 + illr feeds the sigmoid
                            terms.append((ident, illr_c[:, ps_]))
                        for j, (lhsT, rhs) in enumerate(terms):
                            nc.tensor.matmul(
                                ps[:], lhsT[:], rhs,
                                start=(j == 0), stop=(j == len(terms) - 1),
                            )
                        # evac into var: matmuls read the var chunk first, so
                        # this is chunk-local WAR (gathers only read t).
                        # Last two chunks evacuate on DVE: the ACT queue (lns
                        # + tanhs) is the serial tail bottleneck there, while
                        # DVE has drained; earlier chunks stay on ACT so the
                        # DVE product cadence that paces the gathers is not
                        # stretched.
                        if c == NCHUNK - 1:
                            nc.vector.tensor_copy(var[:, vs], ps[:])
                        else:
                            nc.scalar.activation(var[:, vs], ps[:], AF.Copy)
                        if c == NCHUNK - 1:
                            # produce the last t slice straight from PSUM so
                            # the next iteration's gathers do not wait for the
                            # evac chain to finish first
                            if last:
                                nc.scalar.activation(t[:, vs], ps[:], AF.Sigmoid)
                            else:
                                nc.scalar.activation(
                                    t[:, vs], ps[:], AF.Tanh, scale=0.5
                                )
                h = (NCHUNK - 1) * NC
                if last:
                    nc.sync.dma_start(y_d[:, 0:h], t[:, 0:h])
                    nc.sync.dma_start(y_d[:, h:N], t[:, h:N])
    nc.compile()
    return nc


def _pack_idx(check_index_tensor):
    """k-major within node-chunks of NC: for chunk c the flat order is
    [ci[cs,0], ci[cs,1], ..., ci[cs,9]]; wrapped per 16 partitions,
    replicated x8."""
    ci = check_index_tensor.astype(np.int64)  # (N, K)
    cols = []
    for c in range(NCHUNK):
        cc = ci[c * NC : (c + 1) * NC]  # (NC, K)
        flat = np.ascontiguousarray(cc.T.reshape(-1))  # k-major (GI,)
        ch = flat.reshape(GI // 16, 16).T  # (16, 660)
        cols.append(ch)
    w16 = np.concatenate(cols, axis=1)  # (16, 660*8)
    return np.tile(w16, (8, 1)).astype(np.int16)  # (128, 5280)


_CACHE = {}


def kernel(input_llr, w_ch, w_res, check_index_tensor, var_index_tensor):
    input_llr = np.asarray(input_llr, dtype=np.float32)
    w_ch = np.asarray(w_ch, dtype=np.float32)
    w_res = np.asarray(w_res, dtype=np.float32)
    ci = np.asarray(check_index_tensor).astype(np.int64)

    if not (np.all(w_ch == 1.0) and np.all(w_res == 1.0)):
        return _host_reference(input_llr, w_ch, w_res, ci)

    from concourse.bass_utils import run_bass_kernel_spmd

    if "nc" not in _CACHE:
        _CACHE["nc"] = _build_nc()
    nc = _CACHE["nc"]

    idx = _pack_idx(ci)
    ident = np.eye(128, dtype=np.float32)
    nident = -ident
    in_maps = []
    for c in range(NCORES):
        in_maps.append(
            {
                "x": np.ascontiguousarray(input_llr[c * BCORE : (c + 1) * BCORE]),
                "idx": idx,
                "ident": ident,
                "nident": nident,
            }
        )
    trace = bool(_CACHE.get("trace"))
    try:
        res = run_bass_kernel_spmd(nc, in_maps, list(range(NCORES)), trace=trace)
    except Exception:
        if not trace:
            raise
        res = run_bass_kernel_spmd(nc, in_maps, list(range(NCORES)))
    _CACHE["last_exec_time_ns"] = res.exec_time_ns
    out = np.concatenate([res.results[c]["y"] for c in range(NCORES)], axis=0)
    return out.astype(np.float32)
